# revision 6
# baseline (speedup 1.0000x reference)
"""Trainium2 Bass kernel for nn_DGBasedGaussianKLD.

Math (per reference):
  z[b,s,d] = mean[b,d] + eps[b,s,d]*exp(0.5*logvar[b,d])
  For each chunk c (batch split into nc=4 chunks of agg_size=256) and each
  dim d, with samples j = (b_local, s) (8192 of them) and components
  i = the 256 chunk rows:
    log_q_ij = -0.5*((z_j - mu_i)^2 * e^{-lv_i} + lv_i + LOG2PI)
    q_j  = mean_i exp(log_q_ij)
    logq[c,d] = mean_j log q_j
    logp[c,d] = mean_j -0.5*(z_j^2 + LOG2PI)
  out = sum_d mean_c (logq - logp)

Quadrature reformulation (device work 64x smaller than direct eval):
  For fixed (c,d), f(x) = ln sum_i exp(a_i x^2 + b_i x + c_i) is a smooth
  1-D function.  mean_j f(z_j) is computed by evaluating f on a uniform
  B=128-point grid spanning [min z, max z] and combining with Catmull-Rom
  cubic-interpolation weights accumulated from the samples (host-side
  bincounts).  Measured quadrature rel-err on the final scalar: ~3e-6
  (device bf16 numerics dominate at ~1e-4).

The per-(c,d) affine map x = xmid + s*u (u in [-1,1] shared grid) is folded
into the coefficients so the grid operand X is shared by all pairs/cores:
    a' = a s^2,  b' = (2 a xmid + b) s,  c' = a xmid^2 + b xmid + c

Sharding: 128 (c,d) pairs over 8 cores = 16 pairs/core
(core k -> chunk k//2, dims (k%2)*16 .. +16).

Device kernel per core:
  - PE: 8 matmuls, K=8 (split-bf16 rows), N=512: E = X^T W -> PSUM
    [128 grid pts, 16 pairs x 256 comps]
  - ACT: exp over [128, 2048] PSUM -> SBUF bf16 (x2 halves)
  - DVE: bf16 pairwise-add tree 256->32 per pair + 1x tensor_reduce
    -> q[128 pts, 16 pairs] f32
Host: builds X/W operands + quadrature weights (~1M flops), computes
ln q, weighted sums, logp, and the final scalar in float64.
"""

import numpy as np

LOG2PI = float(np.log(2.0 * np.pi))
N_CORES = 8

# Hardcoded problem geometry (see spec): batch=1024, dim_z=32, n_samples=32,
# agg_size=256 -> nchunks=4.
BATCH, DIM_Z, N_SAMPLES, AGG = 1024, 32, 32, 256
NCHUNK = BATCH // AGG           # 4
B = 128                         # grid points per (chunk, dim) pair
NPC = 16                        # pairs per core (4*32 / 8)
NSAMP = AGG * N_SAMPLES         # 8192 samples per chunk

_PROG = None


def _build_program():
    import concourse.bacc as bacc
    import concourse.tile as tile
    from concourse import mybir

    AF = mybir.ActivationFunctionType
    ALU = mybir.AluOpType
    f32 = mybir.dt.float32
    bf16 = mybir.dt.bfloat16

    nc = bacc.Bacc(
        "TRN2", target_bir_lowering=False, debug=False, num_devices=N_CORES
    )
    # Split-bf16 operands (fp32-grade accuracy, bf16 matmul speed):
    # E = u2h*ah + u2l*ah + u2h*al + uh*bh + ul*bh + uh*bl + ch + cl
    # K=8 contraction: no padding, no SBUF zeroing needed.
    # Matmuls alternate between PE row groups 0 and 64 so weight loads
    # overlap in-flight matmuls (concurrent 32-row subarrays).  The W
    # columns are partitioned between the groups and the grid operand X
    # (rows [u2h, u2l, u2h, uh, ul, uh, 1, 1]) is appended to each
    # group's W so one DMA per group delivers everything.
    # w8x rows 8g:8g+8, cols: [blk0 blk1 (round 0) | 128 X | blk2 blk3].
    # Round-0 cols + X ship in a first DMA per group so matmuls start
    # ~1us earlier; round-1 cols follow in a second DMA.
    w8x_d = nc.dram_tensor(
        "w8x", [16, NPC * AGG // 2 + B], bf16, kind="ExternalInput"
    ).ap()
    out_d = nc.dram_tensor("out", [B, NPC], f32, kind="ExternalOutput").ap()
    XCOL = NPC * AGG // 4  # 1024: X operand sits after round-0's blocks
    C1 = XCOL + B          # 1152: round-1 blocks start here

    def blkcol(blk):
        return blk * 512 if blk < 2 else C1 + (blk - 2) * 512

    with tile.TileContext(nc) as tc:
        with (
            tc.tile_pool(name="io", bufs=1) as iop,
            tc.tile_pool(name="ps", bufs=2, space="PSUM") as pp,
            tc.tile_pool(name="ex", bufs=2) as ep,
            tc.tile_pool(name="tree", bufs=9) as tp,
            tc.tile_pool(name="misc", bufs=1) as mp,
        ):
            ws = iop.tile([72, C1 + XCOL], bf16)
            q = mp.tile([B, NPC], f32)
            nc.sync.dma_start(ws[0:8, 0:C1], w8x_d[0:8, 0:C1])
            nc.sync.dma_start(ws[64:72, 0:C1], w8x_d[8:16, 0:C1])
            nc.sync.dma_start(ws[0:8, C1:], w8x_d[0:8, C1:])
            nc.sync.dma_start(ws[64:72, C1:], w8x_d[8:16, C1:])
            # warm the exp activation table while the DMAs are in flight
            warm = mp.tile([128, 1], f32)
            warm2 = mp.tile([128, 1], f32)
            nc.vector.memset(warm[:], 1.0)
            nc.scalar.activation(warm2[:], warm[:], AF.Exp)

            def tree_reduce(ex_ap, npair, qcol):
                # per-pair sum over 256 comps: bf16 pairwise-add tree
                # (2x DVE mode) down to 32/pair, then one 1x reduce.
                h = ex_ap
                w = AGG
                for _ in range(3):  # 256 -> 32 per pair
                    hn = tp.tile([128, npair * (w // 2)], bf16, tag="tree")
                    hg = h.rearrange("p (g k) -> p g k", g=npair)
                    hng = hn[:].rearrange("p (g k) -> p g k", g=npair)
                    nc.vector.tensor_tensor(
                        hng[:, :, :],
                        hg[:, :, 0 : w // 2],
                        hg[:, :, w // 2 : w],
                        ALU.add,
                    )
                    h, w = hn[:], w // 2
                nc.vector.tensor_reduce(
                    q[:, qcol : qcol + npair],
                    h.rearrange("p (g k) -> p g k", g=npair),
                    axis=mybir.AxisListType.X,
                    op=ALU.add,
                )

            for half in range(2):  # 8 pairs per half
                ps = pp.tile([128, 8 * AGG], f32)  # 4 PSUM banks
                for s in range(4):  # 2 pairs per matmul (N=512 = 1 bank)
                    g = s % 2          # PE row group (partition 64*g)
                    blk = 2 * half + s // 2
                    c = blkcol(blk)
                    nc.tensor.matmul(
                        ps[:, s * 512 : (s + 1) * 512],
                        lhsT=ws[64 * g : 64 * g + 8, XCOL : XCOL + B],
                        rhs=ws[64 * g : 64 * g + 8, c : c + 512],
                        start=True,
                        stop=True,
                        tile_position=(64 * g, 0),
                    )
                ex = ep.tile([128, 8 * AGG], bf16)
                if half == 0:
                    nc.scalar.activation(ex[:], ps[:], AF.Exp)
                    tree_reduce(ex[:], 8, 0)
                else:
                    # split exp so the first tree chunk overlaps the
                    # second exp (shorter critical tail)
                    nc.scalar.activation(ex[:, 0:1024], ps[:, 0:1024], AF.Exp)
                    nc.scalar.activation(ex[:, 1024:], ps[:, 1024:], AF.Exp)
                    tree_reduce(ex[:, 0:1024], 4, 8)
                    tree_reduce(ex[:, 1024:], 4, 12)
            nc.sync.dma_start(out_d[:], q[:])

    nc.compile()
    return nc


def _get_program():
    global _PROG
    if _PROG is None:
        _PROG = _build_program()
    return _PROG


def _reference_numpy(mean, logvar, eps, n_samples, agg_size):
    """Exact fallback for unexpected geometry (never hit for the spec case)."""
    batch, dim_z = mean.shape
    if batch % agg_size != 0:
        agg_size = batch
    nchunks = batch // agg_size
    std = np.exp(0.5 * logvar)
    z = mean[:, None, :] + eps * std[:, None, :]
    z2 = z.reshape(nchunks, agg_size * n_samples, dim_z)
    mu = mean.reshape(nchunks, agg_size, 1, dim_z)
    lv = logvar.reshape(nchunks, agg_size, 1, dim_z)
    log_q = -0.5 * (
        (z2[:, None, :, :] - mu) ** 2 * np.exp(-lv) + lv + LOG2PI
    )
    logq = np.log(np.exp(log_q).mean(axis=1)).mean(axis=1)
    logp = (-0.5 * (z2**2 + LOG2PI)).mean(axis=1)
    return np.float32((logq - logp).mean(axis=0).sum(axis=-1))


def _split_bf16(v):
    import ml_dtypes

    bf = ml_dtypes.bfloat16
    hi = v.astype(np.float32).astype(bf)
    lo = (v.astype(np.float32) - hi.astype(np.float32)).astype(bf)
    return hi, lo


def _prep(mean, logvar, eps):
    """Host prep: z, grid ranges, folded split-bf16 coefficients, weights."""
    import ml_dtypes

    bf = ml_dtypes.bfloat16

    # z with the same f32 op order as the reference
    std = np.exp(np.float32(0.5) * logvar)
    z = mean[:, None, :] + eps * std[:, None, :]  # [1024, 32, 32] f32
    z2 = z.reshape(NCHUNK, NSAMP, DIM_Z)

    x0 = z2.min(axis=1).astype(np.float64)  # [nc, dim_z]
    x1 = z2.max(axis=1).astype(np.float64)
    xmid = 0.5 * (x0 + x1)
    s = 0.5 * (x1 - x0)

    mu = mean.astype(np.float64).reshape(NCHUNK, AGG, DIM_Z)
    lv = logvar.astype(np.float64).reshape(NCHUNK, AGG, DIM_Z)
    e = np.exp(-lv)
    a = -0.5 * e                                    # [nc, agg, dim_z]
    b = mu * e
    c = -0.5 * (mu * mu * e + lv + LOG2PI)
    # fold x = xmid + s*u into the quadratic (u in [-1,1])
    a2 = a * (s * s)[:, None, :]
    b2 = (2.0 * a * xmid[:, None, :] + b) * s[:, None, :]
    c2 = (a * xmid[:, None, :] + b) * xmid[:, None, :] + c

    # shared grid operand
    u = -1.0 + 2.0 * np.arange(B) / (B - 1)         # f64 [128]
    u2h, u2l = _split_bf16(u * u)
    uh, ul = _split_bf16(u)
    ones = np.ones(B, dtype=bf)
    x8 = np.stack([u2h, u2l, u2h, uh, ul, uh, ones, ones])  # [8, 128]

    ah, al = _split_bf16(a2)  # [nc, agg, dim_z] each
    bh, bl = _split_bf16(b2)
    ch, cl = _split_bf16(c2)

    in_maps = []
    for core in range(N_CORES):
        cidx, hd = divmod(core, 2)
        d0 = hd * NPC
        # rows [8], dims [pair, comp]
        def pf(v):
            return np.ascontiguousarray(v[cidx, :, d0 : d0 + NPC].T).astype(bf)

        w8 = np.stack([pf(ah), pf(ah), pf(al), pf(bh), pf(bh), pf(bl),
                       pf(ch), pf(cl)])  # [8, NPC, AGG]
        # group/block layout: group g, block b holds pairs
        # p0 = 8*(b//2) + 4*(b%2) + 2g and p0+1.
        # cols per group row-block: [blk0 blk1 | X | blk2 blk3]
        w8x = np.zeros((16, NPC * AGG // 2 + B), dtype=bf)
        for g in range(2):
            for b_ in range(4):
                c0 = b_ * 512 if b_ < 2 else 1152 + (b_ - 2) * 512
                p0 = 8 * (b_ // 2) + 4 * (b_ % 2) + 2 * g
                w8x[8 * g : 8 * g + 8, c0 : c0 + 256] = w8[:, p0]
                w8x[8 * g : 8 * g + 8, c0 + 256 : c0 + 512] = w8[:, p0 + 1]
            w8x[8 * g : 8 * g + 8, 1024:1152] = x8
        in_maps.append({"w8x": w8x})

    # Catmull-Rom quadrature weights per (c,d): [nc, dim_z, B]
    wq = np.zeros((NCHUNK, DIM_Z, B))
    h = (x1 - x0) / (B - 1)                          # [nc, dim_z]
    for ci in range(NCHUNK):
        for d in range(DIM_Z):
            zd = z2[ci, :, d].astype(np.float64)
            t = (zd - x0[ci, d]) / h[ci, d]
            i = np.clip(np.floor(t).astype(np.int64), 0, B - 2)
            fr = t - i
            im1 = np.clip(i - 1, 0, B - 1)
            ip2 = np.clip(i + 2, 0, B - 1)
            f2 = fr * fr
            f3 = f2 * fr
            wq[ci, d] += np.bincount(im1, -0.5 * fr + f2 - 0.5 * f3, minlength=B)
            wq[ci, d] += np.bincount(i, 1.0 - 2.5 * f2 + 1.5 * f3, minlength=B)
            wq[ci, d] += np.bincount(i + 1, 0.5 * fr + 2.0 * f2 - 1.5 * f3,
                                     minlength=B)
            wq[ci, d] += np.bincount(ip2, -0.5 * f2 + 0.5 * f3, minlength=B)

    return in_maps, z2, wq


def kernel(mean, logvar, eps, n_samples, agg_size):
    from concourse.bass_utils import run_bass_kernel_spmd

    mean = np.asarray(mean, dtype=np.float32)
    logvar = np.asarray(logvar, dtype=np.float32)
    eps = np.asarray(eps, dtype=np.float32)
    n_samples = int(n_samples)
    agg_size = int(agg_size)

    if (mean.shape, eps.shape, n_samples, agg_size) != (
        (BATCH, DIM_Z),
        (BATCH, N_SAMPLES, DIM_Z),
        N_SAMPLES,
        AGG,
    ):
        return _reference_numpy(mean, logvar, eps, n_samples, agg_size)

    in_maps, z2, wq = _prep(mean, logvar, eps)

    nc = _get_program()
    res = run_bass_kernel_spmd(nc, in_maps, list(range(N_CORES)))
    global _LAST_RESULTS
    _LAST_RESULTS = res

    # logq[c,d] = sum_b wq[c,d,b] * (ln q[c,d,b] - ln 256) / 8192
    logq = np.zeros((NCHUNK, DIM_Z))
    for core in range(N_CORES):
        cidx, hd = divmod(core, 2)
        qv = res.results[core]["out"].astype(np.float64)  # [B, NPC]
        f = np.log(np.maximum(qv, 1e-300)) - np.log(256.0)
        w = wq[cidx, hd * NPC : (hd + 1) * NPC]           # [NPC, B]
        logq[cidx, hd * NPC : (hd + 1) * NPC] = (
            np.where(w != 0.0, w * f.T, 0.0).sum(axis=1) / NSAMP
        )

    z64 = z2.astype(np.float64)
    logp = (-0.5 * (z64**2 + LOG2PI)).mean(axis=1)        # [nc, dim_z]
    return np.float32(((logq - logp).mean(axis=0)).sum())


# revision 9
# speedup vs baseline: 1.0898x; 1.0898x over previous
"""Trainium2 Bass kernel for nn_DGBasedGaussianKLD.

Math (per reference):
  z[b,s,d] = mean[b,d] + eps[b,s,d]*exp(0.5*logvar[b,d])
  For each chunk c (batch split into nc=4 chunks of agg_size=256) and each
  dim d, with samples j = (b_local, s) (8192 of them) and components
  i = the 256 chunk rows:
    log_q_ij = -0.5*((z_j - mu_i)^2 * e^{-lv_i} + lv_i + LOG2PI)
    q_j  = mean_i exp(log_q_ij)
    logq[c,d] = mean_j log q_j
    logp[c,d] = mean_j -0.5*(z_j^2 + LOG2PI)
  out = sum_d mean_c (logq - logp)

Quadrature reformulation (device work 64x smaller than direct eval):
  For fixed (c,d), f(x) = ln sum_i exp(a_i x^2 + b_i x + c_i) is a smooth
  1-D function.  mean_j f(z_j) is computed by evaluating f on a uniform
  B=128-point grid spanning [min z, max z] and combining with Catmull-Rom
  cubic-interpolation weights accumulated from the samples (host-side
  bincounts).  Measured quadrature rel-err on the final scalar: ~3e-6
  (device bf16 numerics dominate at ~1e-4).

The per-(c,d) affine map x = xmid + s*u (u in [-1,1] shared grid) is folded
into the coefficients so the grid operand X is shared by all pairs/cores:
    a' = a s^2,  b' = (2 a xmid + b) s,  c' = a xmid^2 + b xmid + c

Sharding: 128 (c,d) pairs over 8 cores = 16 pairs/core
(core k -> chunk k//2, dims (k%2)*16 .. +16).

Device kernel per core:
  - PE: 8 matmuls, K=8 (split-bf16 rows), N=512: E = X^T W -> PSUM
    [128 grid pts, 16 pairs x 256 comps]
  - ACT: exp over [128, 2048] PSUM -> SBUF bf16 (x2 halves)
  - DVE: bf16 pairwise-add tree 256->32 per pair + 1x tensor_reduce
    -> q[128 pts, 16 pairs] f32
Host: builds X/W operands + quadrature weights (~1M flops), computes
ln q, weighted sums, logp, and the final scalar in float64.
"""

import numpy as np

LOG2PI = float(np.log(2.0 * np.pi))
N_CORES = 8

# Hardcoded problem geometry (see spec): batch=1024, dim_z=32, n_samples=32,
# agg_size=256 -> nchunks=4.
BATCH, DIM_Z, N_SAMPLES, AGG = 1024, 32, 32, 256
NCHUNK = BATCH // AGG           # 4
B = 128                         # grid points per (chunk, dim) pair
NPC = 16                        # pairs per core (4*32 / 8)
NSAMP = AGG * N_SAMPLES         # 8192 samples per chunk

_PROG = None


def _build_program():
    import concourse.bacc as bacc
    import concourse.tile as tile
    from concourse import mybir

    AF = mybir.ActivationFunctionType
    ALU = mybir.AluOpType
    f32 = mybir.dt.float32
    bf16 = mybir.dt.bfloat16

    nc = bacc.Bacc(
        "TRN2", target_bir_lowering=False, debug=False, num_devices=N_CORES
    )
    # Split-bf16 operands (fp32-grade accuracy, bf16 matmul speed):
    # E = u2h*ah + u2l*ah + u2h*al + uh*bh + ul*bh + uh*bl + ch + cl
    # K=8 contraction: no padding, no SBUF zeroing needed.
    # Matmuls alternate between PE row groups 0 and 64 so weight loads
    # overlap in-flight matmuls (concurrent 32-row subarrays).  The W
    # columns are partitioned between the groups and the grid operand X
    # (rows [u2h, u2l, u2h, uh, ul, uh, 1, 1]) is appended to each
    # group's W so one DMA per group delivers everything.
    # w8x rows 8g:8g+8, cols: [blk0 blk1 (round 0) | 128 X | blk2 blk3].
    # Round-0 cols + X ship in a first DMA per group so matmuls start
    # ~1us earlier; round-1 cols follow in a second DMA.
    w8x_d = nc.dram_tensor(
        "w8x", [16, NPC * AGG // 2 + B], bf16, kind="ExternalInput"
    ).ap()
    # Raw exp values ship back to the host (bf16), which does the
    # 256-component sums in f32 -- no on-device reduction at all, and
    # the output DMAs pipeline behind the exp chunks.
    out_d = nc.dram_tensor(
        "out", [B, NPC * AGG], bf16, kind="ExternalOutput"
    ).ap()
    XCOL = NPC * AGG // 4  # 1024: X operand sits after round-0's blocks
    C1 = XCOL + B          # 1152: round-1 blocks start here

    def blkcol(blk):
        return blk * 512 if blk < 2 else C1 + (blk - 2) * 512

    with tile.TileContext(nc) as tc:
        with (
            tc.tile_pool(name="io", bufs=1) as iop,
            tc.tile_pool(name="ps", bufs=2, space="PSUM") as pp,
            tc.tile_pool(name="ex", bufs=1) as ep,
            tc.tile_pool(name="misc", bufs=1) as mp,
        ):
            ws = iop.tile([72, C1 + XCOL], bf16)
            ex = ep.tile([128, 2 * 8 * AGG], bf16)
            nc.sync.dma_start(ws[0:8, 0:C1], w8x_d[0:8, 0:C1])
            nc.sync.dma_start(ws[64:72, 0:C1], w8x_d[8:16, 0:C1])
            nc.sync.dma_start(ws[0:8, C1:], w8x_d[0:8, C1:])
            nc.sync.dma_start(ws[64:72, C1:], w8x_d[8:16, C1:])
            # warm the exp activation table while the DMAs are in flight
            warm = mp.tile([128, 1], f32)
            warm2 = mp.tile([128, 1], f32)
            nc.vector.memset(warm[:], 1.0)
            nc.scalar.activation(warm2[:], warm[:], AF.Exp)

            for half in range(2):  # 8 pairs per half
                ps = pp.tile([128, 8 * AGG], f32)  # 4 PSUM banks
                for s in range(4):  # 2 pairs per matmul (N=512 = 1 bank)
                    g = s % 2          # PE row group (partition 64*g)
                    blk = 2 * half + s // 2
                    c = blkcol(blk)
                    nc.tensor.matmul(
                        ps[:, s * 512 : (s + 1) * 512],
                        lhsT=ws[64 * g : 64 * g + 8, XCOL : XCOL + B],
                        rhs=ws[64 * g : 64 * g + 8, c : c + 512],
                        start=True,
                        stop=True,
                        tile_position=(64 * g, 0),
                    )
                e0 = half * 2048
                if half == 0:
                    # one big exp; its DMA overlaps the next round
                    nc.scalar.activation(ex[:, e0 : e0 + 2048], ps[:], AF.Exp)
                    nc.sync.dma_start(
                        out_d[:, e0 : e0 + 2048], ex[:, e0 : e0 + 2048]
                    )
                else:
                    # split exp so each chunk's DMA fires sooner
                    for j in range(2):
                        c0 = e0 + j * 1024
                        nc.scalar.activation(
                            ex[:, c0 : c0 + 1024],
                            ps[:, j * 1024 : (j + 1) * 1024],
                            AF.Exp,
                        )
                        nc.sync.dma_start(
                            out_d[:, c0 : c0 + 1024], ex[:, c0 : c0 + 1024]
                        )

    nc.compile()
    return nc


def _get_program():
    global _PROG
    if _PROG is None:
        _PROG = _build_program()
    return _PROG


def _reference_numpy(mean, logvar, eps, n_samples, agg_size):
    """Exact fallback for unexpected geometry (never hit for the spec case)."""
    batch, dim_z = mean.shape
    if batch % agg_size != 0:
        agg_size = batch
    nchunks = batch // agg_size
    std = np.exp(0.5 * logvar)
    z = mean[:, None, :] + eps * std[:, None, :]
    z2 = z.reshape(nchunks, agg_size * n_samples, dim_z)
    mu = mean.reshape(nchunks, agg_size, 1, dim_z)
    lv = logvar.reshape(nchunks, agg_size, 1, dim_z)
    log_q = -0.5 * (
        (z2[:, None, :, :] - mu) ** 2 * np.exp(-lv) + lv + LOG2PI
    )
    logq = np.log(np.exp(log_q).mean(axis=1)).mean(axis=1)
    logp = (-0.5 * (z2**2 + LOG2PI)).mean(axis=1)
    return np.float32((logq - logp).mean(axis=0).sum(axis=-1))


def _split_bf16(v):
    import ml_dtypes

    bf = ml_dtypes.bfloat16
    hi = v.astype(np.float32).astype(bf)
    lo = (v.astype(np.float32) - hi.astype(np.float32)).astype(bf)
    return hi, lo


def _prep(mean, logvar, eps):
    """Host prep: z, grid ranges, folded split-bf16 coefficients, weights."""
    import ml_dtypes

    bf = ml_dtypes.bfloat16

    # z with the same f32 op order as the reference
    std = np.exp(np.float32(0.5) * logvar)
    z = mean[:, None, :] + eps * std[:, None, :]  # [1024, 32, 32] f32
    z2 = z.reshape(NCHUNK, NSAMP, DIM_Z)

    x0 = z2.min(axis=1).astype(np.float64)  # [nc, dim_z]
    x1 = z2.max(axis=1).astype(np.float64)
    xmid = 0.5 * (x0 + x1)
    s = 0.5 * (x1 - x0)

    mu = mean.astype(np.float64).reshape(NCHUNK, AGG, DIM_Z)
    lv = logvar.astype(np.float64).reshape(NCHUNK, AGG, DIM_Z)
    e = np.exp(-lv)
    a = -0.5 * e                                    # [nc, agg, dim_z]
    b = mu * e
    c = -0.5 * (mu * mu * e + lv + LOG2PI)
    # fold x = xmid + s*u into the quadratic (u in [-1,1])
    a2 = a * (s * s)[:, None, :]
    b2 = (2.0 * a * xmid[:, None, :] + b) * s[:, None, :]
    c2 = (a * xmid[:, None, :] + b) * xmid[:, None, :] + c

    # shared grid operand
    u = -1.0 + 2.0 * np.arange(B) / (B - 1)         # f64 [128]
    u2h, u2l = _split_bf16(u * u)
    uh, ul = _split_bf16(u)
    ones = np.ones(B, dtype=bf)
    x8 = np.stack([u2h, u2l, u2h, uh, ul, uh, ones, ones])  # [8, 128]

    ah, al = _split_bf16(a2)  # [nc, agg, dim_z] each
    bh, bl = _split_bf16(b2)
    ch, cl = _split_bf16(c2)

    in_maps = []
    for core in range(N_CORES):
        cidx, hd = divmod(core, 2)
        d0 = hd * NPC
        # rows [8], dims [pair, comp]
        def pf(v):
            return np.ascontiguousarray(v[cidx, :, d0 : d0 + NPC].T).astype(bf)

        w8 = np.stack([pf(ah), pf(ah), pf(al), pf(bh), pf(bh), pf(bl),
                       pf(ch), pf(cl)])  # [8, NPC, AGG]
        # group/block layout: group g, block b holds pairs
        # p0 = 8*(b//2) + 4*(b%2) + 2g and p0+1.
        # cols per group row-block: [blk0 blk1 | X | blk2 blk3]
        w8x = np.zeros((16, NPC * AGG // 2 + B), dtype=bf)
        for g in range(2):
            for b_ in range(4):
                c0 = b_ * 512 if b_ < 2 else 1152 + (b_ - 2) * 512
                p0 = 8 * (b_ // 2) + 4 * (b_ % 2) + 2 * g
                w8x[8 * g : 8 * g + 8, c0 : c0 + 256] = w8[:, p0]
                w8x[8 * g : 8 * g + 8, c0 + 256 : c0 + 512] = w8[:, p0 + 1]
            w8x[8 * g : 8 * g + 8, 1024:1152] = x8
        in_maps.append({"w8x": w8x})

    # Catmull-Rom quadrature weights per (c,d): [nc, dim_z, B]
    wq = np.zeros((NCHUNK, DIM_Z, B))
    h = (x1 - x0) / (B - 1)                          # [nc, dim_z]
    for ci in range(NCHUNK):
        for d in range(DIM_Z):
            zd = z2[ci, :, d].astype(np.float64)
            t = (zd - x0[ci, d]) / h[ci, d]
            i = np.clip(np.floor(t).astype(np.int64), 0, B - 2)
            fr = t - i
            im1 = np.clip(i - 1, 0, B - 1)
            ip2 = np.clip(i + 2, 0, B - 1)
            f2 = fr * fr
            f3 = f2 * fr
            wq[ci, d] += np.bincount(im1, -0.5 * fr + f2 - 0.5 * f3, minlength=B)
            wq[ci, d] += np.bincount(i, 1.0 - 2.5 * f2 + 1.5 * f3, minlength=B)
            wq[ci, d] += np.bincount(i + 1, 0.5 * fr + 2.0 * f2 - 1.5 * f3,
                                     minlength=B)
            wq[ci, d] += np.bincount(ip2, -0.5 * f2 + 0.5 * f3, minlength=B)

    return in_maps, z2, wq


def kernel(mean, logvar, eps, n_samples, agg_size):
    from concourse.bass_utils import run_bass_kernel_spmd

    mean = np.asarray(mean, dtype=np.float32)
    logvar = np.asarray(logvar, dtype=np.float32)
    eps = np.asarray(eps, dtype=np.float32)
    n_samples = int(n_samples)
    agg_size = int(agg_size)

    if (mean.shape, eps.shape, n_samples, agg_size) != (
        (BATCH, DIM_Z),
        (BATCH, N_SAMPLES, DIM_Z),
        N_SAMPLES,
        AGG,
    ):
        return _reference_numpy(mean, logvar, eps, n_samples, agg_size)

    in_maps, z2, wq = _prep(mean, logvar, eps)

    nc = _get_program()
    res = run_bass_kernel_spmd(nc, in_maps, list(range(N_CORES)))
    global _LAST_RESULTS
    _LAST_RESULTS = res

    # logq[c,d] = sum_b wq[c,d,b] * (ln q[c,d,b] - ln 256) / 8192
    logq = np.zeros((NCHUNK, DIM_Z))
    for core in range(N_CORES):
        cidx, hd = divmod(core, 2)
        exv = res.results[core]["out"].astype(np.float32)  # [B, 16*256] bf16
        qv = exv.reshape(B, NPC, AGG).sum(axis=2, dtype=np.float32)
        qv = qv.astype(np.float64)                         # [B, NPC]
        f = np.log(np.maximum(qv, 1e-300)) - np.log(256.0)
        w = wq[cidx, hd * NPC : (hd + 1) * NPC]            # [NPC, B]
        logq[cidx, hd * NPC : (hd + 1) * NPC] = (
            np.where(w != 0.0, w * f.T, 0.0).sum(axis=1) / NSAMP
        )

    z64 = z2.astype(np.float64)
    logp = (-0.5 * (z64**2 + LOG2PI)).mean(axis=1)        # [nc, dim_z]
    return np.float32(((logq - logp).mean(axis=0)).sum())
